# revision 7
# baseline (speedup 1.0000x reference)
"""Dense-MoE (all experts, softmax-gated) Trainium2 kernel.

Math reformulation (per token t):
  s1    = x @ [Wd_cat | Wg]                # one K=768 matmul -> [64 h1 | 8 logits]
  h1b   = s1[:64] + bd_cat
  exp_e = exp(s1[64:72] + bg)              # unnormalized gate
  s2    = h1b @ blockdiag(Wm)              # one K=64 matmul
  g64   = expand(exp)                      # K=8 matmul vs 0/1 matrix
  s3in  = [(s2 + bm) * g64 ; exp]          # [72]
  Z     = exp @ ones                       # K=8, N=1 matmul per 128-token group
  o     = s3in @ [Wu_cat ; bu]             # K=72 matmul
  out   = o / Z                            # softmax normalization folded to the end

Perf design:
  - fp16 end to end on chip (inputs cast + pre-transposed host-side, output
    upcast host-side): halves HBM traffic vs fp32 and removes all on-chip
    transposes, so the PE only runs productive matmuls.
  - PE instruction stream is kept dense (next tile's stage-1 interleaved with
    current tile's stage-3) so the HAM activity monitor holds the PE at
    K=8/8 (2.4 GHz) instead of re-throttling to 1.2 GHz during epilogue gaps.
  - All big DMAs ride the two hardware DGE queues (sync: loads, scalar:
    stores + weights); the gpsimd software queue only serves the warmup
    memset. Stage-3 PSUM tiles span 2 banks so each 128-token group needs a
    single normalization multiply.
  - Data-parallel over tokens, 8 cores, weights replicated.
"""

import numpy as np

B, S, D, E, R = 8, 4096, 768, 8, 8
NCORES = 8
T_CORE = B * S // NCORES          # 4096 tokens per core
TILE_T = 512                      # tokens per compute tile
N_TILES = T_CORE // TILE_T        # 8
EW = E * R                        # 64
KW = EW + E                       # 72
KC = D // 128                     # 6 contraction chunks for stage 1
JC = TILE_T // 128                # 4 token chunks of 128 per tile
XW = KC * TILE_T                  # 3072 packed x columns per tile
OW = JC * D                       # 3072 packed out columns per tile
HD = D // 2                       # 384: stage-3 half width

_CACHE = {}


def _build_and_compile():
    """Build the Bass/Tile program once. Returns compiled nc."""
    from contextlib import ExitStack

    import concourse.bass as bass
    import concourse.tile as tile
    from concourse import bacc, mybir

    f32 = mybir.dt.float32
    f16 = mybir.dt.float16
    AF = mybir.ActivationFunctionType
    ALU = mybir.AluOpType

    nc = bacc.Bacc("TRN2", target_bir_lowering=False, debug=False, num_devices=NCORES)

    NW = KC * KW + EW + EW + D + 1               # 1329 packed fp16 weight columns
    x_d = nc.dram_tensor("x", [N_TILES * 128, XW], f16, kind="ExternalInput").ap()
    wp_d = nc.dram_tensor("wpack", [128, NW], f16, kind="ExternalInput").ap()
    bias_d = nc.dram_tensor("bias", [EW, 4], f32, kind="ExternalInput").ap()
    out_d = nc.dram_tensor("out", [N_TILES * 128, OW], f16, kind="ExternalOutput").ap()

    # tile i, partition p: x_v[i, p, c*512 + t] = x[token i*512+t, d=c*128+p]
    x_v = x_d.rearrange("(i p) w -> i p w", p=128)
    # tile i, partition p: out_v[i, p, j*768 + d] = out[token i*512+j*128+p, d]
    out_v = out_d.rearrange("(i p) w -> i p w", p=128)

    with tile.TileContext(nc) as tc, ExitStack() as ctx:
        const = ctx.enter_context(tc.tile_pool(name="const", bufs=1))
        xin = ctx.enter_context(tc.tile_pool(name="xin", bufs=4))
        mid_p = ctx.enter_context(tc.tile_pool(name="mid", bufs=2))
        outp = ctx.enter_context(tc.tile_pool(name="outp", bufs=3))
        small = ctx.enter_context(tc.tile_pool(name="small", bufs=2))
        # PSUM budget (8 banks): s1/warm 1 + s2 1 + g64 1 + z 1 + s3 2x2 = 8
        s1p = ctx.enter_context(tc.tile_pool(name="s1p", bufs=1, space="PSUM"))
        s2p = ctx.enter_context(tc.tile_pool(name="s2p", bufs=1, space="PSUM"))
        g64p = ctx.enter_context(tc.tile_pool(name="g64p", bufs=1, space="PSUM"))
        zpp = ctx.enter_context(tc.tile_pool(name="zpp", bufs=1, space="PSUM"))
        s3ap = ctx.enter_context(tc.tile_pool(name="s3ap", bufs=2, space="PSUM"))

        # Warmup memset is the only gpsimd (software-DGE) work; both hardware
        # DGE queues start immediately: x(0) on sync, weights on scalar.
        warm_src = const.tile([128, TILE_T], f16, name="warm_src")
        nc.gpsimd.memset(warm_src[:], 0.0)

        x_sb0 = xin.tile([128, XW], f16, name="x_sb0", tag="x")
        nc.sync.dma_start(x_sb0[:], x_v[0])

        wp = const.tile([128, NW], f16, name="wp")
        nc.scalar.dma_start(wp[:], wp_d)
        bias_sb = const.tile([EW, 4], f32, name="bias_sb")
        nc.scalar.dma_start(bias_sb[:], bias_d)

        c0 = 0
        w1_sb = wp[:, c0:c0 + KC * KW]; c0 += KC * KW
        wm_sb = wp[0:EW, c0:c0 + EW]; c0 += EW
        e8_sb = wp[EW:KW, c0:c0 + EW]; c0 += EW
        w3_sb = wp[0:KW, c0:c0 + D]; c0 += D
        ones_sb = wp[EW:KW, c0:c0 + 1]; c0 += 1
        bd_sb = bias_sb[:, 0:1]
        bm_sb = bias_sb[:, 1:2]
        bg_sb = bias_sb[0:E, 2:3]

        # HAM pre-warm: ~3.4us of fp16 matmuls on memset garbage (no DMA
        # dependency) so the PE is at K=8/8 (2.4GHz) when tile 0 arrives.
        warm_ps = s1p.tile([128, TILE_T], f32, name="warm_ps", tag="s1")
        for _k in range(8):
            nc.tensor.matmul(
                warm_ps[:], warm_src[:, 0:128], warm_src[:],
                start=True, stop=True,
            )

        x_sbs, h1bs, s3ins, rcs, outs, s3ps = {}, {}, {}, {}, {}, {}

        def load(i):
            if i == 0:
                x_sbs[0] = x_sb0
                return
            x_sb = xin.tile([128, XW], f16, name="x_sb", tag="x")
            nc.sync.dma_start(x_sb[:], x_v[i])
            x_sbs[i] = x_sb

        def zmm_recip(i):
            """Per-128-token-group Z = sum_e exp_e via 4 tiny matmuls, then
            one batched reciprocal -> rc[128, 4]."""
            s3in = s3ins[i]
            zps = zpp.tile([128, JC], f32, name="zps", tag="z")
            for j in range(JC):
                nc.tensor.matmul(
                    zps[:, j:j + 1],
                    s3in[EW:KW, j * 128:(j + 1) * 128],
                    ones_sb,
                    start=True, stop=True,
                )
            rc = small.tile([128, JC], f32, name="rc", tag="rc")
            nc.vector.reciprocal(rc[:], zps[:])
            rcs[i] = rc

        def s1mm(i):
            x_sb = x_sbs.pop(i)
            s1 = s1p.tile([KW, TILE_T], f32, name="s1", tag="s1")
            for c in range(KC):
                nc.tensor.matmul(
                    s1[:],
                    w1_sb[:, c * KW:(c + 1) * KW],
                    x_sb[:, c * TILE_T:(c + 1) * TILE_T],
                    start=(c == 0),
                    stop=(c == KC - 1),
                )
            return s1

        def epi1(i, s1):
            h1b = mid_p.tile([EW, TILE_T], f16, name="h1b", tag="h1b")
            s3in = mid_p.tile([KW, TILE_T], f16, name="s3in", tag="s3in")
            nc.scalar.activation(s3in[EW:KW, :], s1[EW:KW, :], AF.Exp, bias=bg_sb)
            nc.vector.tensor_scalar_add(h1b[:], s1[0:EW, :], bd_sb)
            h1bs[i], s3ins[i] = h1b, s3in
            return s1

        def gmm(i):
            s3in = s3ins[i]
            g64_ps = g64p.tile([EW, TILE_T], f32, name="g64_ps", tag="g64p")
            nc.tensor.matmul(
                g64_ps[:], e8_sb, s3in[EW:KW, :], start=True, stop=True
            )
            g64 = mid_p.tile([EW, TILE_T], f32, name="g64", tag="g64")
            nc.scalar.copy(g64[:], g64_ps[:])
            return g64

        def s2mm(i):
            h1b = h1bs.pop(i)
            s2 = s2p.tile([EW, TILE_T], f32, name="s2", tag="s2")
            nc.tensor.matmul(s2[:], wm_sb, h1b[:], start=True, stop=True)
            return s2

        def stt(i, s1g):
            s2, g64 = s1g
            s3in = s3ins[i]
            nc.vector.scalar_tensor_tensor(
                s3in[0:EW, :], s2[:], bm_sb, g64[:],
                op0=ALU.add, op1=ALU.mult,
            )

        def s3mm(i, j):
            s3in = s3ins[i]
            lhsT = s3in[:, j * 128:(j + 1) * 128]
            # split at 512 so each matmul's PSUM write stays inside one bank
            s3w = s3ap.tile([128, D], f32, name="s3w", tag="s3")
            nc.tensor.matmul(s3w[:, 0:512], lhsT, w3_sb[:, 0:512], start=True, stop=True)
            nc.tensor.matmul(s3w[:, 512:D], lhsT, w3_sb[:, 512:D], start=True, stop=True)
            if j == 0:
                outs[i] = outp.tile([128, OW], f16, name="out_sb", tag="out")
            s3ps[(i, j)] = s3w

        def muls(i, j, eng):
            s3w = s3ps.pop((i, j))
            rc, out_sb = rcs[i], outs[i]
            if eng == "act":
                nc.scalar.mul(out_sb[:, j * D:(j + 1) * D], s3w[:], rc[:, j:j + 1])
            else:
                nc.vector.tensor_scalar_mul(
                    out_sb[:, j * D:(j + 1) * D], s3w[:], rc[:, j:j + 1]
                )

        def store(i):
            out_sb = outs.pop(i)
            rcs.pop(i)
            s3ins.pop(i)
            nc.scalar.dma_start(out_v[i], out_sb[:])

        def store_chunk(i, j, eng):
            out_sb = outs[i]
            dma = nc.scalar.dma_start if eng == "act" else nc.sync.dma_start
            dma(out_v[i, :, j * D:(j + 1) * D], out_sb[:, j * D:(j + 1) * D])

        # Software-pipelined emission. Iteration i runs tile i's front half
        # (stage 1/2, gating) interleaved with tile i-1's back half (stage 3,
        # normalization, store) so the PE queue never drains. The final two
        # tiles' back halves run interleaved after the loop with chunked
        # stores on both hardware DMA queues so the output drain overlaps
        # the compute tail.
        load(0)
        load(1)
        load(2)
        load(3)
        for i in range(N_TILES):
            p = i - 1
            back = i > 0 and i < N_TILES - 1
            if i > 0:
                zmm_recip(p)
            s1 = s1mm(i)
            if i + 4 < N_TILES:
                load(i + 4)
            epi1(i, s1)
            if back:
                s3mm(p, 0)
                muls(p, 0, "act")
                s3mm(p, 1)
                muls(p, 1, "dve")
            g64 = gmm(i)
            s2 = s2mm(i)
            if back:
                s3mm(p, 2)
                muls(p, 2, "act")
                s3mm(p, 3)
                muls(p, 3, "dve")
                stt(i, (s2, g64))
                store(p)
            else:
                stt(i, (s2, g64))
        pa, pb = N_TILES - 2, N_TILES - 1
        zmm_recip(pb)
        for j in range(JC):
            s3mm(pa, j)
            s3mm(pb, j)
            muls(pa, j, "act" if j % 2 == 0 else "dve")
            muls(pb, j, "dve" if j % 2 == 0 else "act")
            store_chunk(pa, j, "act")
            store_chunk(pb, j, "sync")
        for i in (pa, pb):
            outs.pop(i)
            rcs.pop(i)
            s3ins.pop(i)

    nc.compile()
    return nc


def _pack_host_inputs(Wd, bd, Wm, bm, Wu, bu, Wg, bg):
    """Repack the tiny weights into the on-chip layouts (host-side, ~100KB)."""
    f = np.float32
    W1 = np.concatenate(
        [np.ascontiguousarray(Wd.transpose(1, 0, 2)).reshape(D, EW), Wg], axis=1
    ).astype(f)                                   # [768, 72]
    w1p = np.ascontiguousarray(
        W1.reshape(KC, 128, KW).transpose(1, 0, 2)
    ).reshape(128, KC * KW)                       # [128, 432]; chunk c at cols c*72

    wmbd = np.zeros((EW, EW), f)
    for e in range(E):
        wmbd[e * R:(e + 1) * R, e * R:(e + 1) * R] = Wm[e]

    e8 = np.kron(np.eye(E, dtype=f), np.ones((1, R), f))   # [8, 64]

    w3e = np.zeros((KW, D), f)
    w3e[:EW, :] = Wu.reshape(EW, D)
    w3e[EW:, :] = bu

    NW = KC * KW + EW + EW + D + 1
    wpack = np.zeros((128, NW), f)
    c0 = 0
    wpack[:, c0:c0 + KC * KW] = w1p; c0 += KC * KW
    wpack[0:EW, c0:c0 + EW] = wmbd; c0 += EW
    wpack[EW:KW, c0:c0 + EW] = e8; c0 += EW
    wpack[0:KW, c0:c0 + D] = w3e; c0 += D
    wpack[EW:KW, c0] = 1.0; c0 += 1

    bias = np.zeros((EW, 4), f)
    bias[:, 0] = bd.reshape(EW)
    bias[:, 1] = bm.reshape(EW)
    bias[0:E, 2] = bg.reshape(E)
    return {"wpack": wpack.astype(np.float16), "bias": bias}


def _pack_x_core(xc16):
    """[T_CORE, D] fp16 -> [N_TILES*128, XW] with x[p, c*512+t] layout."""
    return np.ascontiguousarray(
        xc16.reshape(N_TILES, TILE_T, KC, 128).transpose(0, 3, 2, 1)
    ).reshape(N_TILES * 128, XW)


def _unpack_out_core(oc16):
    """[N_TILES*128, OW] fp16 -> [T_CORE, D] fp32."""
    return (
        oc16.reshape(N_TILES, 128, JC, D)
        .transpose(0, 2, 1, 3)
        .reshape(T_CORE, D)
        .astype(np.float32)
    )


def _run(inputs, trace=False, **kw):
    from concourse import bass_utils

    if "nc" not in _CACHE:
        _CACHE["nc"] = _build_and_compile()
    nc = _CACHE["nc"]

    x16 = np.asarray(inputs["x"]).astype(np.float16).reshape(B * S, D)
    w = _pack_host_inputs(
        *(np.asarray(inputs[k], dtype=np.float32)
          for k in ["Wd", "bd", "Wm", "bm", "Wu", "bu", "Wg", "bg"])
    )
    in_maps = [
        {"x": _pack_x_core(x16[i * T_CORE:(i + 1) * T_CORE]), **w}
        for i in range(NCORES)
    ]
    res = bass_utils.run_bass_kernel_spmd(
        nc, in_maps, core_ids=list(range(NCORES)), trace=trace, **kw
    )
    out = np.concatenate(
        [_unpack_out_core(res.results[i]["out"]) for i in range(NCORES)], axis=0
    ).reshape(B, S, D)
    return out, res


def kernel(**inputs) -> np.ndarray:
    out, _ = _run(inputs)
    return out


# revision 9
# speedup vs baseline: 1.2226x; 1.2226x over previous
"""Dense-MoE (all experts, softmax-gated) Trainium2 kernel.

Math reformulation (per token t):
  s1    = x @ [Wd_cat | Wg]                # one K=768 matmul -> [64 h1 | 8 logits]
  h1b   = s1[:64] + bd_cat
  exp_e = exp(s1[64:72] + bg)              # unnormalized gate
  s2    = h1b @ blockdiag(Wm)              # one K=64 matmul
  g64   = expand(exp)                      # K=8 matmul vs 0/1 matrix
  s3in  = [(s2 + bm) * g64 ; exp]          # [72]
  Z     = exp @ ones                       # K=8, N=1 matmul per 128-token group
  o     = s3in @ [Wu_cat ; bu]             # K=72 matmul
  out   = o / Z                            # softmax normalization folded to the end

Perf design:
  - fp16 end to end on chip (inputs cast + pre-transposed host-side, output
    upcast host-side): halves HBM traffic vs fp32 and removes all on-chip
    transposes, so the PE only runs productive matmuls.
  - PE instruction stream is kept dense (next tile's stage-1 interleaved with
    current tile's stage-3) so the HAM activity monitor holds the PE at
    K=8/8 (2.4 GHz) instead of re-throttling to 1.2 GHz during epilogue gaps.
  - All big DMAs ride the two hardware DGE queues (sync: loads, scalar:
    stores + weights); the gpsimd software queue only serves the warmup
    memset. Stage-3 PSUM tiles span 2 banks so each 128-token group needs a
    single normalization multiply.
  - Data-parallel over tokens, 8 cores, weights replicated.
"""

import numpy as np

B, S, D, E, R = 8, 4096, 768, 8, 8
NCORES = 8
T_CORE = B * S // NCORES          # 4096 tokens per core
TILE_T = 512                      # tokens per compute tile
N_TILES = T_CORE // TILE_T        # 8
EW = E * R                        # 64
KW = EW + E                       # 72
KC = D // 128                     # 6 contraction chunks for stage 1
JC = TILE_T // 128                # 4 token chunks of 128 per tile
XW = KC * TILE_T                  # 3072 packed x columns per tile
OW = JC * D                       # 3072 packed out columns per tile
HD = D // 2                       # 384: stage-3 half width

_CACHE = {}


def _build_and_compile():
    """Build the Bass/Tile program once. Returns compiled nc."""
    from contextlib import ExitStack

    import concourse.bass as bass
    import concourse.tile as tile
    from concourse import bacc, mybir

    f32 = mybir.dt.float32
    f16 = mybir.dt.float16
    AF = mybir.ActivationFunctionType
    ALU = mybir.AluOpType

    nc = bacc.Bacc("TRN2", target_bir_lowering=False, debug=False, num_devices=NCORES)

    NW = KC * KW + EW + EW + D + 1               # 1329 packed fp16 weight columns
    x_d = nc.dram_tensor("x", [N_TILES * 128, XW], f16, kind="ExternalInput").ap()
    wp_d = nc.dram_tensor("wpack", [128, NW], f16, kind="ExternalInput").ap()
    bias_d = nc.dram_tensor("bias", [EW, 4], f32, kind="ExternalInput").ap()
    out_d = nc.dram_tensor("out", [N_TILES * 128, OW], f16, kind="ExternalOutput").ap()

    # tile i, partition p: x_v[i, p, c*512 + t] = x[token i*512+t, d=c*128+p]
    x_v = x_d.rearrange("(i p) w -> i p w", p=128)
    # tile i, partition p: out_v[i, p, j*768 + d] = out[token i*512+j*128+p, d]
    out_v = out_d.rearrange("(i p) w -> i p w", p=128)

    with tile.TileContext(nc) as tc, ExitStack() as ctx:
        const = ctx.enter_context(tc.tile_pool(name="const", bufs=1))
        xin = ctx.enter_context(tc.tile_pool(name="xin", bufs=4))
        mid_p = ctx.enter_context(tc.tile_pool(name="mid", bufs=2))
        outp = ctx.enter_context(tc.tile_pool(name="outp", bufs=3))
        small = ctx.enter_context(tc.tile_pool(name="small", bufs=2))
        # PSUM budget (8 banks): s1/warm 1 + s2 1 + g64 1 + z 1 + s3 2x2 = 8
        s1p = ctx.enter_context(tc.tile_pool(name="s1p", bufs=1, space="PSUM"))
        s2p = ctx.enter_context(tc.tile_pool(name="s2p", bufs=1, space="PSUM"))
        g64p = ctx.enter_context(tc.tile_pool(name="g64p", bufs=1, space="PSUM"))
        zpp = ctx.enter_context(tc.tile_pool(name="zpp", bufs=1, space="PSUM"))
        s3ap = ctx.enter_context(tc.tile_pool(name="s3ap", bufs=2, space="PSUM"))

        # Warmup memset is the only gpsimd (software-DGE) work; both hardware
        # DGE queues start immediately: x(0) on sync, weights on scalar.
        warm_src = const.tile([128, TILE_T], f16, name="warm_src")
        nc.gpsimd.memset(warm_src[:], 0.0)

        x_sb0 = xin.tile([128, XW], f16, name="x_sb0", tag="x")
        nc.sync.dma_start(x_sb0[:], x_v[0])

        wp = const.tile([128, NW], f16, name="wp")
        nc.scalar.dma_start(wp[:], wp_d)
        bias_sb = const.tile([EW, 4], f32, name="bias_sb")
        nc.scalar.dma_start(bias_sb[:], bias_d)

        c0 = 0
        w1_sb = wp[:, c0:c0 + KC * KW]; c0 += KC * KW
        wm_sb = wp[0:EW, c0:c0 + EW]; c0 += EW
        e8_sb = wp[EW:KW, c0:c0 + EW]; c0 += EW
        w3_sb = wp[0:KW, c0:c0 + D]; c0 += D
        ones_sb = wp[EW:KW, c0:c0 + 1]; c0 += 1
        bd_sb = bias_sb[:, 0:1]
        bm_sb = bias_sb[:, 1:2]
        bg_sb = bias_sb[0:E, 2:3]

        # HAM pre-warm: ~3.4us of fp16 matmuls on memset garbage (no DMA
        # dependency) so the PE is at K=8/8 (2.4GHz) when tile 0 arrives.
        warm_ps = s1p.tile([128, TILE_T], f32, name="warm_ps", tag="s1")
        for _k in range(8):
            nc.tensor.matmul(
                warm_ps[:], warm_src[:, 0:128], warm_src[:],
                start=True, stop=True,
            )

        x_sbs, h1bs, s3ins, rcs, outs, s3ps = {}, {}, {}, {}, {}, {}

        def load(i):
            if i == 0:
                x_sbs[0] = x_sb0
                return
            x_sb = xin.tile([128, XW], f16, name="x_sb", tag="x")
            nc.sync.dma_start(x_sb[:], x_v[i])
            x_sbs[i] = x_sb

        def zmm_recip(i):
            """Per-128-token-group Z = sum_e exp_e via 4 tiny matmuls, then
            one batched reciprocal -> rc[128, 4]."""
            s3in = s3ins[i]
            zps = zpp.tile([128, JC], f32, name="zps", tag="z")
            for j in range(JC):
                nc.tensor.matmul(
                    zps[:, j:j + 1],
                    s3in[EW:KW, j * 128:(j + 1) * 128],
                    ones_sb,
                    start=True, stop=True,
                )
            rc = small.tile([128, JC], f32, name="rc", tag="rc")
            nc.vector.reciprocal(rc[:], zps[:])
            rcs[i] = rc

        def s1mm(i):
            x_sb = x_sbs.pop(i)
            s1 = s1p.tile([KW, TILE_T], f32, name="s1", tag="s1")
            for c in range(KC):
                nc.tensor.matmul(
                    s1[:],
                    w1_sb[:, c * KW:(c + 1) * KW],
                    x_sb[:, c * TILE_T:(c + 1) * TILE_T],
                    start=(c == 0),
                    stop=(c == KC - 1),
                )
            return s1

        def epi1(i, s1):
            h1b = mid_p.tile([EW, TILE_T], f16, name="h1b", tag="h1b")
            s3in = mid_p.tile([KW, TILE_T], f16, name="s3in", tag="s3in")
            nc.scalar.activation(s3in[EW:KW, :], s1[EW:KW, :], AF.Exp, bias=bg_sb)
            nc.vector.tensor_scalar_add(h1b[:], s1[0:EW, :], bd_sb)
            h1bs[i], s3ins[i] = h1b, s3in
            return s1

        def gmm(i):
            s3in = s3ins[i]
            g64_ps = g64p.tile([EW, TILE_T], f32, name="g64_ps", tag="g64p")
            nc.tensor.matmul(
                g64_ps[:], e8_sb, s3in[EW:KW, :], start=True, stop=True
            )
            g64 = mid_p.tile([EW, TILE_T], f32, name="g64", tag="g64")
            nc.scalar.copy(g64[:], g64_ps[:])
            return g64

        def s2mm(i):
            h1b = h1bs.pop(i)
            s2 = s2p.tile([EW, TILE_T], f32, name="s2", tag="s2")
            nc.tensor.matmul(s2[:], wm_sb, h1b[:], start=True, stop=True)
            return s2

        def stt(i, s1g):
            s2, g64 = s1g
            s3in = s3ins[i]
            nc.vector.scalar_tensor_tensor(
                s3in[0:EW, :], s2[:], bm_sb, g64[:],
                op0=ALU.add, op1=ALU.mult,
            )

        def s3mm(i, j):
            s3in = s3ins[i]
            lhsT = s3in[:, j * 128:(j + 1) * 128]
            # split at 512 so each matmul's PSUM write stays inside one bank
            s3w = s3ap.tile([128, D], f32, name="s3w", tag="s3")
            nc.tensor.matmul(s3w[:, 0:512], lhsT, w3_sb[:, 0:512], start=True, stop=True)
            nc.tensor.matmul(s3w[:, 512:D], lhsT, w3_sb[:, 512:D], start=True, stop=True)
            if j == 0:
                outs[i] = outp.tile([128, OW], f16, name="out_sb", tag="out")
            s3ps[(i, j)] = s3w

        def muls(i, j, eng):
            s3w = s3ps.pop((i, j))
            rc, out_sb = rcs[i], outs[i]
            if eng == "act":
                nc.scalar.mul(out_sb[:, j * D:(j + 1) * D], s3w[:], rc[:, j:j + 1])
            else:
                nc.vector.tensor_scalar_mul(
                    out_sb[:, j * D:(j + 1) * D], s3w[:], rc[:, j:j + 1]
                )

        def store_head(i):
            # chunk 0 rides the (shared) hardware DGE pipe with the loads...
            out_sb = outs[i]
            nc.scalar.dma_start(out_v[i, :, 0:D], out_sb[:, 0:D])

        def store_rest(i):
            # ...chunks 1-3 ride the independent gpsimd software-DGE pipe, so
            # the two DMA paths drain a tile in parallel (~220 + ~133 GB/s).
            out_sb = outs.pop(i)
            rcs.pop(i)
            s3ins.pop(i)
            nc.gpsimd.dma_start(out_v[i, :, D:OW], out_sb[:, D:OW])

        def store_chunk(i, j, eng):
            out_sb = outs[i]
            dma = nc.scalar.dma_start if eng == "act" else nc.sync.dma_start
            dma(out_v[i, :, j * D:(j + 1) * D], out_sb[:, j * D:(j + 1) * D])

        # Software-pipelined emission. Iteration i runs tile i's front half
        # (stage 1/2, gating) interleaved with tile i-1's back half (stage 3,
        # normalization, store) so the PE queue never drains. The final two
        # tiles' back halves run interleaved after the loop with chunked
        # stores on both hardware DMA queues so the output drain overlaps
        # the compute tail.
        load(0)
        load(1)
        load(2)
        load(3)
        for i in range(N_TILES):
            p = i - 1
            back = i > 0 and i < N_TILES - 1
            if i > 0:
                zmm_recip(p)
            s1 = s1mm(i)
            if i + 4 < N_TILES:
                load(i + 4)
            epi1(i, s1)
            if back:
                s3mm(p, 0)
                muls(p, 0, "act")
                store_head(p)
                s3mm(p, 1)
                muls(p, 1, "dve")
            g64 = gmm(i)
            s2 = s2mm(i)
            if back:
                s3mm(p, 2)
                muls(p, 2, "act")
                s3mm(p, 3)
                muls(p, 3, "dve")
                stt(i, (s2, g64))
                store_rest(p)
            else:
                stt(i, (s2, g64))
        # Tail: the hw pipe is free of loads now, so it takes 5 of the 8
        # remaining chunks (pa on scalar, pb j0 on sync) and the sw pipe the
        # other 3 — both finish with the tail compute.
        pa, pb = N_TILES - 2, N_TILES - 1
        zmm_recip(pb)
        for j in range(JC):
            s3mm(pa, j)
            s3mm(pb, j)
            muls(pa, j, "act" if j % 2 == 0 else "dve")
            muls(pb, j, "dve" if j % 2 == 0 else "act")
            store_chunk(pa, j, "act")
            if j == 0:
                store_chunk(pb, 0, "sync")
        out_pb = outs[pb]
        nc.gpsimd.dma_start(out_v[pb, :, D:OW], out_pb[:, D:OW])
        for i in (pa, pb):
            outs.pop(i)
            rcs.pop(i)
            s3ins.pop(i)

    nc.compile()
    return nc


def _pack_host_inputs(Wd, bd, Wm, bm, Wu, bu, Wg, bg):
    """Repack the tiny weights into the on-chip layouts (host-side, ~100KB)."""
    f = np.float32
    W1 = np.concatenate(
        [np.ascontiguousarray(Wd.transpose(1, 0, 2)).reshape(D, EW), Wg], axis=1
    ).astype(f)                                   # [768, 72]
    w1p = np.ascontiguousarray(
        W1.reshape(KC, 128, KW).transpose(1, 0, 2)
    ).reshape(128, KC * KW)                       # [128, 432]; chunk c at cols c*72

    wmbd = np.zeros((EW, EW), f)
    for e in range(E):
        wmbd[e * R:(e + 1) * R, e * R:(e + 1) * R] = Wm[e]

    e8 = np.kron(np.eye(E, dtype=f), np.ones((1, R), f))   # [8, 64]

    w3e = np.zeros((KW, D), f)
    w3e[:EW, :] = Wu.reshape(EW, D)
    w3e[EW:, :] = bu

    NW = KC * KW + EW + EW + D + 1
    wpack = np.zeros((128, NW), f)
    c0 = 0
    wpack[:, c0:c0 + KC * KW] = w1p; c0 += KC * KW
    wpack[0:EW, c0:c0 + EW] = wmbd; c0 += EW
    wpack[EW:KW, c0:c0 + EW] = e8; c0 += EW
    wpack[0:KW, c0:c0 + D] = w3e; c0 += D
    wpack[EW:KW, c0] = 1.0; c0 += 1

    bias = np.zeros((EW, 4), f)
    bias[:, 0] = bd.reshape(EW)
    bias[:, 1] = bm.reshape(EW)
    bias[0:E, 2] = bg.reshape(E)
    return {"wpack": wpack.astype(np.float16), "bias": bias}


def _pack_x_core(xc16):
    """[T_CORE, D] fp16 -> [N_TILES*128, XW] with x[p, c*512+t] layout."""
    return np.ascontiguousarray(
        xc16.reshape(N_TILES, TILE_T, KC, 128).transpose(0, 3, 2, 1)
    ).reshape(N_TILES * 128, XW)


def _unpack_out_core(oc16):
    """[N_TILES*128, OW] fp16 -> [T_CORE, D] fp32."""
    return (
        oc16.reshape(N_TILES, 128, JC, D)
        .transpose(0, 2, 1, 3)
        .reshape(T_CORE, D)
        .astype(np.float32)
    )


def _run(inputs, trace=False, **kw):
    from concourse import bass_utils

    if "nc" not in _CACHE:
        _CACHE["nc"] = _build_and_compile()
    nc = _CACHE["nc"]

    x16 = np.asarray(inputs["x"]).astype(np.float16).reshape(B * S, D)
    w = _pack_host_inputs(
        *(np.asarray(inputs[k], dtype=np.float32)
          for k in ["Wd", "bd", "Wm", "bm", "Wu", "bu", "Wg", "bg"])
    )
    in_maps = [
        {"x": _pack_x_core(x16[i * T_CORE:(i + 1) * T_CORE]), **w}
        for i in range(NCORES)
    ]
    res = bass_utils.run_bass_kernel_spmd(
        nc, in_maps, core_ids=list(range(NCORES)), trace=trace, **kw
    )
    out = np.concatenate(
        [_unpack_out_core(res.results[i]["out"]) for i in range(NCORES)], axis=0
    ).reshape(B, S, D)
    return out, res


def kernel(**inputs) -> np.ndarray:
    out, _ = _run(inputs)
    return out
